# revision 34
# baseline (speedup 1.0000x reference)
"""Trainium2 Bass kernel for nn_CapsuleLayer (conv capsule layer with dynamic routing).

Full (unsharded) inputs in, full output out. Sharding: data-parallel over the
num_capsules axis A=32 -> 8 cores x 4 capsules each (x windows replicated).

Per-core layout: the 4x576 (capsule, position) rows are flattened and packed
into 18 units of exactly 128 partition-rows (units may straddle a capsule
boundary; handled as 2 segments). Per unit:

  phase A (PE):  priors via 18 block-diag matmuls (fp16 in, f32 PSUM),
                 moving layout (d-major) so the ACT copy lands priors
                 transposed: P1T[p, d, n] fp16 in SBUF.
                 s0 numerator via a dense 18-matmul PSUM chain (wde).
  routing (DVE/ACT):
    g = sum_d P1T[p,d,:] * out[p,d]:  16 tensor_scalar muls (4x DVE perf
        mode: fp16, contiguous, SBUF) + pairwise-add tree (2x mode).
    softmax: tensor_reduce max (negate) -> ACT exp(bias=-max, accum=sum).
    s = sum_n e * P1T: one broadcast tensor_tensor mul (2x) + halving add
        tree over n (2x) + final small reduce.
    squash without ACT-table swaps: sqrt(x) = exp(0.5*ln(x)) so the whole
        kernel stays on the natural_log_exp table (copy/exp/ln share it;
        a Sqrt would cost a 1283ns table reload per use).
  Units are emitted phase-A-sequentially but routing is interleaved in
  pairs so DVE never stalls on ACT round-trips.
"""
import os
import numpy as np

import concourse.bass as bass
import concourse.bacc as bacc
import concourse.mybir as mybir
import concourse.tile as tile
from concourse.bass_utils import run_bass_kernel_spmd

# problem constants (hardcoded per contract)
K = 3
B, Ci, H, Wd, Cin = 4, 32, 14, 14, 8
A, N, D = 32, 288, 16
w = 12
P = B * w * w           # 576 positions
G = 16                  # route nodes per PE chunk
CH = N // G             # 18 chunks; G*Cin = 128 = contraction per chunk
AA = A // 8             # capsules per core
NU = (AA * P) // 128    # 18 units of 128 (a,p) rows
GRP = 3                 # psum groups per unit in phase A
CPG = CH // GRP         # 6 chunks per group

F32 = mybir.dt.float32
F16 = mybir.dt.float16
AL = mybir.AluOpType
AF = mybir.ActivationFunctionType
AX = mybir.AxisListType

# s-side implementation:
#   "a" = pure-DVE broadcast-mul + add-tree
#   "c" = PE-transpose path, EXT copies on ACT (loses on HW: copy cost)
#   "d" = PE-transpose path, one big 2x DVE copy per iter
S_MODE = os.environ.get("KERNEL_SMODE", "a")
# how many of the 16 per-d g-side muls run on DVE / ACT / GPSIMD
G_SPLIT = tuple(int(x) for x in
                os.environ.get("KERNEL_GSPLIT", "6,5,5").split(","))

LAST_RESULT = None

_prog_cache = {}


def _slab_slot(u):
    """xw slab slot for unit u: rows of unit u are xw[p(u,r)] with
    p(u,r) = (128*u + r) mod 576; 9 precomputed alignments cover all units."""
    um = u % 9
    if um <= 3:
        return um          # p = 128*um + r
    if um == 4:
        return 8           # p = (512 + r) mod 576  (the mixed alignment)
    return 4 + (um - 5)    # p = 64 + 128*(um-5) + r


def _segments(u):
    """Unit u covers flattened (a,p) rows [128u, 128u+128).
    Returns [(a, p0, rowofs, cnt)] (1 or 2 segments)."""
    segs = []
    r = u * 128
    end = r + 128
    while r < end:
        a = r // P
        p0 = r % P
        cnt = min(end - r, P - p0)
        segs.append((a, p0, r - u * 128, cnt))
        r += cnt
    return segs


def _build_program():
    key = ("v3", S_MODE)
    if key in _prog_cache:
        return _prog_cache[key]

    nc = bacc.Bacc()
    xwt_d = nc.dram_tensor("xwt", [128, CH, P], F16, kind="ExternalInput")
    wbd_d = nc.dram_tensor("wbd", [AA, 128, CH, D * G], F16, kind="ExternalInput")
    wde_d = nc.dram_tensor("wde", [128, AA, CH, D], F16, kind="ExternalInput")
    bunit_d = nc.dram_tensor("bunit", [128, NU, D], F32, kind="ExternalInput")
    if S_MODE in ("c", "d"):
        xws_d = nc.dram_tensor("xws", [128, 9, CH, Cin, G], F16,
                               kind="ExternalInput")
        wst_d = nc.dram_tensor("wst", [128, AA, CH, D], F16,
                               kind="ExternalInput")
    out_d = nc.dram_tensor("out", [AA, P, D], F32, kind="ExternalOutput")

    # phase-A PSUM grouping: mode c needs PSUM banks for the transpose path,
    # so use smaller groups there (3 chunks -> 2 banks x 2 bufs).
    cpg = {"c": 4, "d": 4}.get(S_MODE, CPG)
    grpn = CH // cpg

    with tile.TileContext(nc) as tc:
        with (
            tc.tile_pool(name="const", bufs=1) as cp,
            tc.tile_pool(name="wbdp", bufs=2) as wp,
            tc.tile_pool(name="p1t", bufs=4) as pp,
            tc.tile_pool(name="big", bufs=2) as tp,
            tc.tile_pool(name="lg", bufs=2) as lp,
            tc.tile_pool(name="sm", bufs=3) as sp,
            tc.tile_pool(name="psum_g", bufs=2, space="PSUM") as qp,
            tc.tile_pool(name="psum_s", bufs=2, space="PSUM") as qs,
            tc.tile_pool(name="psum_x", bufs=1, space="PSUM") as qx,
        ):
            # Pre-place the one ACT table that covers every function we use
            # (copy/exp/ln) so the lowering pass doesn't ping-pong between
            # the exp-only and ln-only tables (1283ns per reload).
            nc.scalar.add_instruction(mybir.InstLoadActFuncSet(
                name=nc.get_next_instruction_name(),
                act_func_set_id=6,  # natural_log_exp_and_others
                ins=[], outs=[]))

            # ---- input DMAs, ordered by first use: xwt group 0 and the
            # first capsule's weights gate the very first matmuls; the xw
            # slabs (5.3MB) are only needed once routing iter-1 starts, and
            # in unit order 0,1,2,3,8(mixed),4..7.
            wbd_tiles = {}

            def get_wbd(a):
                if a not in wbd_tiles:
                    t = wp.tile([128, CH, D * G], F16, tag="wbd")
                    nc.sync.dma_start(t[:], wbd_d[a])
                    wbd_tiles[a] = t
                return wbd_tiles[a]

            xwt = cp.tile([128, CH, P], F16)
            nc.sync.dma_start(xwt[:, 0:CPG, :], xwt_d[:, 0:CPG, :])
            get_wbd(0)
            for gi in range(1, GRP):
                nc.sync.dma_start(xwt[:, gi * CPG:(gi + 1) * CPG, :],
                                  xwt_d[:, gi * CPG:(gi + 1) * CPG, :])
            wde = cp.tile([128, AA, CH, D], F16)
            nc.sync.dma_start(wde[:], wde_d[:])
            bunit = cp.tile([128, NU, D], F32)
            nc.sync.dma_start(bunit[:], bunit_d[:])
            if S_MODE in ("c", "d"):
                wst = cp.tile([128, AA, CH, D], F16)
                nc.sync.dma_start(wst[:], wst_d[:])
                ident = cp.tile([128, 128], F16)
                from concourse.masks import make_identity
                make_identity(nc, ident[:])
                xws = cp.tile([128, 9, CH, Cin, G], F16)
                for sl in (0, 1, 2, 3, 8, 4, 5, 6, 7):
                    nc.sync.dma_start(xws[:, sl], xws_d[:, sl])

            def phase_a(u):
                segs = _segments(u)
                for (a, _, _, _) in segs:
                    get_wbd(a)
                p1t = pp.tile([128, D, N], F16, tag="p1t")
                ch0 = 0
                while ch0 < CH:
                    gsz = min(cpg, CH - ch0)
                    pg = qp.tile([128, cpg, D * G], F32, tag="pg")
                    for cl in range(gsz):
                        ch = ch0 + cl
                        for (a, p0, ro, cnt) in segs:
                            nc.tensor.matmul(
                                pg[ro:ro + cnt, cl, :],
                                xwt[:, ch, p0:p0 + cnt],
                                wbd_tiles[a][:, ch, :],
                                start=True, stop=True)
                    # PSUM (f32, (ch, d, g)) -> SBUF fp16 P1T[:, d, n-range]
                    nc.scalar.copy(
                        p1t[:, :, ch0 * G:(ch0 + gsz) * G]
                        .rearrange("p d (c g) -> p d c g", g=G),
                        pg[:, 0:gsz].rearrange("p c (d g) -> p d c g", g=G))
                    ch0 += gsz
                return (p1t,)

            def s0_chain(u, segs):
                # s0 numerator; emitted at routing start (not in phase_a) so
                # its PSUM-buffer WAR never blocks the PE queue ahead of the
                # s-side matmuls it depends on.
                # psv slots: 0 = s0 numerator, 1 = iter1 s, 2 = iter2 s
                # (padded to a full 2KB PSUM bank so pool packing stays
                #  bank-aligned; unaligned psum tiles alias accumulation
                #  groups across banks)
                psv = qs.tile([128, 32, D], F32, tag="psv")
                for (a, p0, ro, cnt) in segs:
                    for ch in range(CH):
                        nc.tensor.matmul(
                            psv[ro:ro + cnt, 0, :],
                            xwt[:, ch, p0:p0 + cnt],
                            wde[:, a, ch, :],
                            start=(ch == 0), stop=(ch == CH - 1))
                return psv

            def squash_pre(s, sq):
                """sn = |s|^2 and r = 1/(1+sn); the sum and add run on the
                (otherwise idle) GPSIMD, the reciprocal must stay on DVE."""
                junk = sp.tile([128, D], F32, tag="sqjunk")
                sn = sp.tile([128, 1], F32, tag="sn" + sq)
                nc.vector.scalar_tensor_tensor(
                    out=junk[:], in0=s[:], scalar=1.0, in1=s[:],
                    op0=AL.mult, op1=AL.mult, accum_out=sn[:])
                u1 = sp.tile([128, 1], F32, tag="u1")
                nc.vector.tensor_scalar_add(u1[:], sn[:], 1.0)
                r = sp.tile([128, 1], F32, tag="r" + sq)
                nc.vector.reciprocal(r[:], u1[:])
                return sn, r

            def squash_act(sn, sq):
                """ACT part: rt = sqrt(sn) = exp(0.5*ln(sn)); stays on the
                ln/exp table (no table reload)."""
                t = sp.tile([128, 1], F32, tag="t")
                nc.scalar.activation(t[:], sn[:], AF.Ln)
                rt = sp.tile([128, 1], F32, tag="rt" + sq)
                nc.scalar.activation(rt[:], t[:], AF.Exp, scale=0.5)
                return rt

            def squash_post(s, rt, r, sq):
                """out = s * sqrt(sn)/(1+sn); tiny ops, kept off DVE."""
                f = sp.tile([128, 1], F32, tag="f")
                nc.vector.tensor_mul(f[:], rt[:], r[:])
                o = sp.tile([128, D], F32, tag="o" + sq)
                nc.vector.tensor_scalar_mul(o[:], s[:], f[:])
                return o

            def g_chain(p1t, ov, lg_out, lg_prev):
                """lg_out = sum_d p1t[:,d,:]*ov[:,d] (+ lg_prev): 16 per-d
                scalar muls load-balanced across DVE (tensor_scalar @4x),
                ACT (activation Copy with scale=AP) and GPSIMD, then a
                pairwise-add tree on DVE at 2x."""
                tmp = tp.tile([128, D, N], F16, tag="gtmp")
                tr1 = tp.tile([128, D // 2, N], F16, tag="gtr1")
                nd, na, ng = G_SPLIT
                for d in range(D):
                    if d < nd:
                        nc.vector.tensor_scalar_mul(
                            tmp[:, d, :], p1t[:, d, :], ov[:, d:d + 1])
                    elif d < nd + na:
                        nc.scalar.activation(
                            tmp[:, d, :], p1t[:, d, :], AF.Copy,
                            scale=ov[:, d:d + 1])
                    else:
                        # Pool engine can't take AP scalars (TensorScalarPtr
                        # is DVE-only); emulate with a broadcast tensor_tensor
                        nc.gpsimd.tensor_mul(
                            tmp[:, d, :], p1t[:, d, :],
                            ov[:, d:d + 1].broadcast_to([128, N]))
                nc.vector.tensor_add(tr1[:, 0:8, :], tmp[:, 0:8, :],
                                     tmp[:, 8:16, :])
                nc.vector.tensor_add(tmp[:, 0:4, :], tr1[:, 0:4, :],
                                     tr1[:, 4:8, :])
                nc.vector.tensor_add(tr1[:, 0:2, :], tmp[:, 0:2, :],
                                     tmp[:, 2:4, :])
                if lg_prev is None:
                    nc.vector.tensor_add(lg_out[:], tr1[:, 0, :], tr1[:, 1, :])
                else:
                    nc.vector.tensor_add(tr1[:, 2, :], tr1[:, 0, :],
                                         tr1[:, 1, :])
                    nc.vector.tensor_add(lg_out[:], tr1[:, 2, :], lg_prev[:])
                return tmp, tr1

            def s_side_a(p1t, e, tmp, tr1):
                """sr[:,d] = sum_n e * p1t[:,d,:]: one 2x broadcast mul +
                halving add tree over n, reusing g-side scratch."""
                w2 = tp.tile([128, D, N], F16, tag="w2")
                nc.vector.tensor_mul(
                    w2[:], p1t[:],
                    e[:, None, :].broadcast_to([128, D, N]))
                nc.vector.tensor_add(tmp[:, :, 0:144], w2[:, :, 0:144],
                                     w2[:, :, 144:288])
                nc.vector.tensor_add(w2[:, :, 0:72], tmp[:, :, 0:72],
                                     tmp[:, :, 72:144])
                nc.vector.tensor_add(tmp[:, :, 0:36], w2[:, :, 0:36],
                                     w2[:, :, 36:72])
                nc.vector.tensor_add(w2[:, :, 0:18], tmp[:, :, 0:18],
                                     tmp[:, :, 18:36])
                sr = sp.tile([128, D], F32, tag="sr")
                nc.vector.tensor_reduce(out=sr[:], in_=w2[:, :, 0:18],
                                        axis=AX.X, op=AL.add)
                return sr

            def s_side_c(u, segs, e, psv, slot):
                """s numerator on the PE: EX = e*xw (one 2x DVE mul), PE
                transposes EX chunks through PSUM (GPSIMD copies them back
                as fp16 stationaries), then 18 shared-weight matmuls
                accumulate sum_{n,c} EX^T * W into psv[:, slot, :]."""
                slab = xws[:, _slab_slot(u)]
                ex = tp.tile([128, CH, Cin, G], F16, tag="ex")
                for (a, p0, ro, cnt) in segs:
                    nc.vector.tensor_mul(
                        ex[ro:ro + cnt], slab[ro:ro + cnt],
                        e[ro:ro + cnt]
                        .rearrange("p (c2 g) -> p c2 g", g=G)[:, :, None, :]
                        .broadcast_to([cnt, CH, Cin, G]))
                exts = tp.tile([128, CH, 128], F16, tag="exts")
                if S_MODE == "d":
                    # transposes in two bank-aligned PSUM groups of 9, each
                    # drained by one 2x DVE copy
                    for gi in range(2):
                        exq_t = qx.tile([128, 16, 128], F16, tag="exq")
                        for cl in range(9):
                            nc.tensor.transpose(
                                exq_t[:, cl, :],
                                ex[:, gi * 9 + cl].rearrange("p c g -> p (c g)"),
                                ident[:])
                        nc.vector.tensor_copy(
                            exts[:, gi * 9:(gi + 1) * 9, :],
                            exq_t[:, 0:9, :])
                else:
                    for gi in range(3):
                        exq_t = qx.tile([128, 8, 128], F16, tag="exq")
                        for cl in range(6):
                            ch = gi * 6 + cl
                            nc.tensor.transpose(
                                exq_t[:, cl, :],
                                ex[:, ch].rearrange("p c g -> p (c g)"),
                                ident[:])
                        nc.scalar.copy(exts[:, gi * 6:(gi + 1) * 6, :],
                                       exq_t[:, 0:6, :])
                for (a, p0, ro, cnt) in segs:
                    for ch in range(CH):
                        nc.tensor.matmul(
                            psv[ro:ro + cnt, slot, :],
                            exts[:, ch, ro:ro + cnt],
                            wst[:, a, ch, :],
                            start=(ch == 0), stop=(ch == CH - 1))
                return psv[:, slot, :]

            def routing_gen(u, p1t):
                segs = _segments(u)
                psv = s0_chain(u, segs)
                bu = bunit[:, u, :]
                # ---- iter 0 (uniform probs; s0 from the PE chain)
                s = sp.tile([128, D], F32, tag="s0")
                nc.vector.scalar_tensor_tensor(
                    out=s[:], in0=psv[:, 0, :], scalar=1.0 / N, in1=bu,
                    op0=AL.mult, op1=AL.add)
                sn, r = squash_pre(s, "0")
                yield
                rt = squash_act(sn, "0")
                yield
                ov = squash_post(s, rt, r, "0")

                lg_prev = None
                for it in (1, 2):
                    sq = str(it)
                    lg = lp.tile([128, N], F16, tag="lg" + sq)
                    tmp, tr1 = g_chain(p1t, ov, lg, lg_prev)
                    lg_prev = lg
                    nmx = sp.tile([128, 1], F32, tag="nmx")
                    nc.vector.tensor_reduce(out=nmx[:], in_=lg[:], axis=AX.X,
                                            op=AL.max, negate=True)
                    yield
                    e = sp.tile([128, N], F16, tag="e")
                    se = sp.tile([128, 1], F32, tag="se")
                    nc.scalar.activation(e[:], lg[:], AF.Exp, bias=nmx[:],
                                         scale=1.0, accum_out=se[:])
                    yield
                    rc = sp.tile([128, 1], F32, tag="rc")
                    nc.vector.reciprocal(rc[:], se[:])
                    if S_MODE in ("c", "d"):
                        sr = s_side_c(u, segs, e, psv, it)
                    else:
                        sr = s_side_a(p1t, e, tmp, tr1)
                    s = sp.tile([128, D], F32, tag="s" + sq)
                    nc.vector.scalar_tensor_tensor(
                        out=s[:], in0=sr[:], scalar=rc[:], in1=bu,
                        op0=AL.mult, op1=AL.add)
                    sn, r = squash_pre(s, sq)
                    yield
                    rt = squash_act(sn, sq)
                    yield
                    ov = squash_post(s, rt, r, sq)

                for (a, p0, ro, cnt) in segs:
                    nc.sync.dma_start(out_d[a, p0:p0 + cnt, :],
                                      ov[ro:ro + cnt, :])
                yield

            # ---- emit: phase A one pair ahead (keeps the PE from head-of-
            # line blocking behind routing-side PE work), routing zipped
            # across each pair so DVE never stalls on ACT/PE round-trips.
            pa = {}

            def ensure_pa(u):
                if u < NU and u not in pa:
                    pa[u] = phase_a(u)

            ensure_pa(0)
            ensure_pa(1)
            for j in range(0, NU, 2):
                ensure_pa(j + 2)
                ensure_pa(j + 3)
                g0 = routing_gen(j, *pa[j])
                g1 = routing_gen(j + 1, *pa[j + 1])
                # stagger the pair by two stages so the two units hit their
                # cross-engine waits out of phase (a zipped pair otherwise
                # stalls simultaneously)
                next(g0)
                next(g0)
                alive = [g0, g1]
                while alive:
                    nxt = []
                    for g in alive:
                        try:
                            next(g)
                            nxt.append(g)
                        except StopIteration:
                            pass
                    alive = nxt

    nc.finalize()
    _prog_cache[key] = nc
    return nc


def _host_prep(x, route_weights, bias):
    x = np.ascontiguousarray(x, dtype=np.float32)
    Wfull = np.ascontiguousarray(route_weights, dtype=np.float32)
    bias = np.ascontiguousarray(bias, dtype=np.float32)

    # im2col: xw[p, n, c], node ordering (ci, ki, kj) as in torch .view
    xw = np.empty((B, w, w, Ci, K, K, Cin), np.float32)
    for ki in range(K):
        for kj in range(K):
            xw[:, :, :, :, ki, kj, :] = (
                x[:, :, ki:ki + w, kj:kj + w, :].transpose(0, 2, 3, 1, 4))
    xw = xw.reshape(P, N, Cin)

    # xwt[(g,c), ch, p]
    xw4 = xw.reshape(P, CH, G, Cin)
    xwt_h = np.ascontiguousarray(
        xw4.transpose(2, 3, 1, 0).reshape(128, CH, P)).astype(np.float16)

    Wn = Wfull.reshape(A, CH, G, Cin, D)
    # block-diag moving weights, (d, g)-major columns:
    # wbd[a, (g,c), ch, (d, g')] = W[a, ch, g, c, d] iff g' == g
    wbd_full = np.zeros((A, G, Cin, CH, D, G), np.float32)
    for g in range(G):
        # Wn[:, :, g, :, :]: [A, CH, Cin, D] -> [A, Cin, CH, D]
        wbd_full[:, g, :, :, :, g] = Wn[:, :, g, :, :].transpose(0, 2, 1, 3)
    wbd_h = wbd_full.reshape(A, 128, CH, D * G).astype(np.float16)

    # wde[(g,c), a, ch, d]
    wde_h = np.ascontiguousarray(
        Wn.transpose(2, 3, 0, 1, 4).reshape(128, A, CH, D)).astype(np.float16)

    if S_MODE in ("c", "d"):
        # xw in (p, ch, c, g) order, staged at 9 partition alignments so any
        # unit's 128 rows are one contiguous slab slice
        xw_cng = np.ascontiguousarray(
            xw.reshape(P, CH, G, Cin).transpose(0, 1, 3, 2))  # [P, CH, c, g]
        rows = np.arange(128)
        xws_h = np.zeros((128, 9, CH, Cin, G), np.float16)
        for q in range(4):
            xws_h[:, q] = xw_cng[128 * q + rows]
        for kk in range(4):
            xws_h[:, 4 + kk] = xw_cng[64 + 128 * kk + rows]
        xws_h[:, 8] = xw_cng[(512 + rows) % P]
        # wst[(c,g), a, ch, d] — (c,g) row order matches EX's flattening
        wst_h = np.ascontiguousarray(
            Wn.transpose(3, 2, 0, 1, 4).reshape(128, A, CH, D)).astype(np.float16)

    in_maps = []
    for k in range(8):
        a0 = k * AA
        # per-unit per-row bias: rows are flattened (a_local, p)
        bunit_h = np.empty((128, NU, D), np.float32)
        for u in range(NU):
            rows = np.arange(u * 128, u * 128 + 128)
            bunit_h[:, u, :] = bias[a0 + rows // P]
        im = {
            "xwt": xwt_h,
            "wbd": np.ascontiguousarray(wbd_h[a0:a0 + AA]),
            "wde": np.ascontiguousarray(wde_h[:, a0:a0 + AA]),
            "bunit": bunit_h,
        }
        if S_MODE in ("c", "d"):
            im["xws"] = xws_h
            im["wst"] = np.ascontiguousarray(wst_h[:, a0:a0 + AA])
        in_maps.append(im)
    return in_maps


def kernel(x, route_weights, bias):
    global LAST_RESULT
    nc = _build_program()
    in_maps = _host_prep(x, route_weights, bias)
    trace = bool(os.environ.get("KERNEL_TRACE"))
    res = run_bass_kernel_spmd(nc, in_maps, list(range(8)), trace=trace)
    LAST_RESULT = res
    full = np.stack([res.results[k]["out"] for k in range(8)])  # [8, AA, P, D]
    full = full.reshape(A, B, w, w, D)
    return np.ascontiguousarray(full.transpose(1, 0, 2, 3, 4))


# revision 35
# speedup vs baseline: 1.1821x; 1.1821x over previous
"""Trainium2 Bass kernel for nn_CapsuleLayer (conv capsule layer with dynamic routing).

Full (unsharded) inputs in, full output out. Sharding: data-parallel over the
num_capsules axis A=32 -> 8 cores x 4 capsules each (x windows replicated).

Per-core layout: the 4x576 (capsule, position) rows are flattened and packed
into 18 units of exactly 128 partition-rows (units may straddle a capsule
boundary; handled as 2 segments). Per unit:

  phase A (PE):  priors via 18 block-diag matmuls (fp16 in, f32 PSUM),
                 moving layout (d-major) so the ACT copy lands priors
                 transposed: P1T[p, d, n] fp16 in SBUF.
                 s0 numerator via a dense 18-matmul PSUM chain (wde).
  routing (DVE/ACT):
    g = sum_d P1T[p,d,:] * out[p,d]:  16 tensor_scalar muls (4x DVE perf
        mode: fp16, contiguous, SBUF) + pairwise-add tree (2x mode).
    softmax: tensor_reduce max (negate) -> ACT exp(bias=-max, accum=sum).
    s = sum_n e * P1T: one broadcast tensor_tensor mul (2x) + halving add
        tree over n (2x) + final small reduce.
    squash without ACT-table swaps: sqrt(x) = exp(0.5*ln(x)) so the whole
        kernel stays on the natural_log_exp table (copy/exp/ln share it;
        a Sqrt would cost a 1283ns table reload per use).
  Units are emitted phase-A-sequentially but routing is interleaved in
  pairs so DVE never stalls on ACT round-trips.
"""
import os
import numpy as np

import concourse.bass as bass
import concourse.bacc as bacc
import concourse.mybir as mybir
import concourse.tile as tile
from concourse.bass_utils import run_bass_kernel_spmd

# problem constants (hardcoded per contract)
K = 3
B, Ci, H, Wd, Cin = 4, 32, 14, 14, 8
A, N, D = 32, 288, 16
w = 12
P = B * w * w           # 576 positions
G = 16                  # route nodes per PE chunk
CH = N // G             # 18 chunks; G*Cin = 128 = contraction per chunk
AA = A // 8             # capsules per core
NU = (AA * P) // 128    # 18 units of 128 (a,p) rows
GRP = 3                 # psum groups per unit in phase A
CPG = CH // GRP         # 6 chunks per group

F32 = mybir.dt.float32
F16 = mybir.dt.float16
AL = mybir.AluOpType
AF = mybir.ActivationFunctionType
AX = mybir.AxisListType

# s-side implementation:
#   "a" = pure-DVE broadcast-mul + add-tree
#   "c" = PE-transpose path, EXT copies on ACT (loses on HW: copy cost)
#   "d" = PE-transpose path, one big 2x DVE copy per iter
S_MODE = os.environ.get("KERNEL_SMODE", "a")
# how many of the 16 per-d g-side muls run on DVE / ACT / GPSIMD
G_SPLIT = tuple(int(x) for x in
                os.environ.get("KERNEL_GSPLIT", "6,5,5").split(","))

LAST_RESULT = None

_prog_cache = {}


def _slab_slot(u):
    """xw slab slot for unit u: rows of unit u are xw[p(u,r)] with
    p(u,r) = (128*u + r) mod 576; 9 precomputed alignments cover all units."""
    um = u % 9
    if um <= 3:
        return um          # p = 128*um + r
    if um == 4:
        return 8           # p = (512 + r) mod 576  (the mixed alignment)
    return 4 + (um - 5)    # p = 64 + 128*(um-5) + r


def _segments(u):
    """Unit u covers flattened (a,p) rows [128u, 128u+128).
    Returns [(a, p0, rowofs, cnt)] (1 or 2 segments)."""
    segs = []
    r = u * 128
    end = r + 128
    while r < end:
        a = r // P
        p0 = r % P
        cnt = min(end - r, P - p0)
        segs.append((a, p0, r - u * 128, cnt))
        r += cnt
    return segs


def _build_program():
    key = ("v3", S_MODE)
    if key in _prog_cache:
        return _prog_cache[key]

    nc = bacc.Bacc()
    xwt_d = nc.dram_tensor("xwt", [128, CH, P], F16, kind="ExternalInput")
    wbd_d = nc.dram_tensor("wbd", [AA, 128, CH, D * G], F16, kind="ExternalInput")
    wde_d = nc.dram_tensor("wde", [128, AA, CH, D], F16, kind="ExternalInput")
    bunit_d = nc.dram_tensor("bunit", [128, NU, D], F32, kind="ExternalInput")
    if S_MODE in ("c", "d"):
        xws_d = nc.dram_tensor("xws", [128, 9, CH, Cin, G], F16,
                               kind="ExternalInput")
        wst_d = nc.dram_tensor("wst", [128, AA, CH, D], F16,
                               kind="ExternalInput")
    out_d = nc.dram_tensor("out", [AA, P, D], F32, kind="ExternalOutput")

    # phase-A PSUM grouping: mode c needs PSUM banks for the transpose path,
    # so use smaller groups there (3 chunks -> 2 banks x 2 bufs).
    cpg = {"c": 4, "d": 4}.get(S_MODE, CPG)
    grpn = CH // cpg

    with tile.TileContext(nc) as tc:
        with (
            tc.tile_pool(name="const", bufs=1) as cp,
            tc.tile_pool(name="wbdp", bufs=2) as wp,
            tc.tile_pool(name="p1t", bufs=4) as pp,
            tc.tile_pool(name="big", bufs=2) as tp,
            tc.tile_pool(name="lg", bufs=2) as lp,
            tc.tile_pool(name="sm", bufs=3) as sp,
            tc.tile_pool(name="psum_g", bufs=2, space="PSUM") as qp,
            tc.tile_pool(name="psum_s", bufs=2, space="PSUM") as qs,
            tc.tile_pool(name="psum_x", bufs=1, space="PSUM") as qx,
        ):
            # Pre-place the one ACT table that covers every function we use
            # (copy/exp/ln) so the lowering pass doesn't ping-pong between
            # the exp-only and ln-only tables (1283ns per reload).
            nc.scalar.add_instruction(mybir.InstLoadActFuncSet(
                name=nc.get_next_instruction_name(),
                act_func_set_id=6,  # natural_log_exp_and_others
                ins=[], outs=[]))

            # ---- input DMAs, ordered by first use: xwt group 0 and the
            # first capsule's weights gate the very first matmuls; the xw
            # slabs (5.3MB) are only needed once routing iter-1 starts, and
            # in unit order 0,1,2,3,8(mixed),4..7.
            wbd_tiles = {}

            def get_wbd(a):
                if a not in wbd_tiles:
                    t = wp.tile([128, CH, D * G], F16, tag="wbd")
                    nc.sync.dma_start(t[:], wbd_d[a])
                    wbd_tiles[a] = t
                return wbd_tiles[a]

            xwt = cp.tile([128, CH, P], F16)
            nc.sync.dma_start(xwt[:, 0:CPG, :], xwt_d[:, 0:CPG, :])
            get_wbd(0)
            for gi in range(1, GRP):
                nc.sync.dma_start(xwt[:, gi * CPG:(gi + 1) * CPG, :],
                                  xwt_d[:, gi * CPG:(gi + 1) * CPG, :])
            wde = cp.tile([128, AA, CH, D], F16)
            nc.sync.dma_start(wde[:], wde_d[:])
            bunit = cp.tile([128, NU, D], F32)
            nc.sync.dma_start(bunit[:], bunit_d[:])
            if S_MODE in ("c", "d"):
                wst = cp.tile([128, AA, CH, D], F16)
                nc.sync.dma_start(wst[:], wst_d[:])
                ident = cp.tile([128, 128], F16)
                from concourse.masks import make_identity
                make_identity(nc, ident[:])
                xws = cp.tile([128, 9, CH, Cin, G], F16)
                for sl in (0, 1, 2, 3, 8, 4, 5, 6, 7):
                    nc.sync.dma_start(xws[:, sl], xws_d[:, sl])

            def phase_a(u):
                segs = _segments(u)
                for (a, _, _, _) in segs:
                    get_wbd(a)
                p1t = pp.tile([128, D, N], F16, tag="p1t")
                ch0 = 0
                while ch0 < CH:
                    gsz = min(cpg, CH - ch0)
                    pg = qp.tile([128, cpg, D * G], F32, tag="pg")
                    for cl in range(gsz):
                        ch = ch0 + cl
                        for (a, p0, ro, cnt) in segs:
                            nc.tensor.matmul(
                                pg[ro:ro + cnt, cl, :],
                                xwt[:, ch, p0:p0 + cnt],
                                wbd_tiles[a][:, ch, :],
                                start=True, stop=True)
                    # PSUM (f32, (ch, d, g)) -> SBUF fp16 P1T[:, d, n-range]
                    nc.scalar.copy(
                        p1t[:, :, ch0 * G:(ch0 + gsz) * G]
                        .rearrange("p d (c g) -> p d c g", g=G),
                        pg[:, 0:gsz].rearrange("p c (d g) -> p d c g", g=G))
                    ch0 += gsz
                psv = s0_chain(u, segs)
                return (p1t, psv)

            def s0_chain(u, segs):
                # s0 numerator; emitted at routing start (not in phase_a) so
                # its PSUM-buffer WAR never blocks the PE queue ahead of the
                # s-side matmuls it depends on.
                # psv slots: 0 = s0 numerator, 1 = iter1 s, 2 = iter2 s
                # (padded to a full 2KB PSUM bank so pool packing stays
                #  bank-aligned; unaligned psum tiles alias accumulation
                #  groups across banks)
                psv = qs.tile([128, 32, D], F32, tag="psv")
                for (a, p0, ro, cnt) in segs:
                    for ch in range(CH):
                        nc.tensor.matmul(
                            psv[ro:ro + cnt, 0, :],
                            xwt[:, ch, p0:p0 + cnt],
                            wde[:, a, ch, :],
                            start=(ch == 0), stop=(ch == CH - 1))
                return psv

            def squash_pre(s, sq):
                """sn = |s|^2 and r = 1/(1+sn); the sum and add run on the
                (otherwise idle) GPSIMD, the reciprocal must stay on DVE."""
                junk = sp.tile([128, D], F32, tag="sqjunk")
                sn = sp.tile([128, 1], F32, tag="sn" + sq)
                nc.vector.scalar_tensor_tensor(
                    out=junk[:], in0=s[:], scalar=1.0, in1=s[:],
                    op0=AL.mult, op1=AL.mult, accum_out=sn[:])
                u1 = sp.tile([128, 1], F32, tag="u1")
                nc.vector.tensor_scalar_add(u1[:], sn[:], 1.0)
                r = sp.tile([128, 1], F32, tag="r" + sq)
                nc.vector.reciprocal(r[:], u1[:])
                return sn, r

            def squash_act(sn, sq):
                """ACT part: rt = sqrt(sn) = exp(0.5*ln(sn)); stays on the
                ln/exp table (no table reload)."""
                t = sp.tile([128, 1], F32, tag="t")
                nc.scalar.activation(t[:], sn[:], AF.Ln)
                rt = sp.tile([128, 1], F32, tag="rt" + sq)
                nc.scalar.activation(rt[:], t[:], AF.Exp, scale=0.5)
                return rt

            def squash_post(s, rt, r, sq):
                """out = s * sqrt(sn)/(1+sn); tiny ops, kept off DVE."""
                f = sp.tile([128, 1], F32, tag="f")
                nc.vector.tensor_mul(f[:], rt[:], r[:])
                o = sp.tile([128, D], F32, tag="o" + sq)
                nc.vector.tensor_scalar_mul(o[:], s[:], f[:])
                return o

            def g_chain(p1t, ov, lg_out, lg_prev):
                """lg_out = sum_d p1t[:,d,:]*ov[:,d] (+ lg_prev): 16 per-d
                scalar muls load-balanced across DVE (tensor_scalar @4x),
                ACT (activation Copy with scale=AP) and GPSIMD, then a
                pairwise-add tree on DVE at 2x."""
                tmp = tp.tile([128, D, N], F16, tag="gtmp")
                tr1 = tp.tile([128, D // 2, N], F16, tag="gtr1")
                nd, na, ng = G_SPLIT
                for d in range(D):
                    if d < nd:
                        nc.vector.tensor_scalar_mul(
                            tmp[:, d, :], p1t[:, d, :], ov[:, d:d + 1])
                    elif d < nd + na:
                        nc.scalar.activation(
                            tmp[:, d, :], p1t[:, d, :], AF.Copy,
                            scale=ov[:, d:d + 1])
                    else:
                        # Pool engine can't take AP scalars (TensorScalarPtr
                        # is DVE-only); emulate with a broadcast tensor_tensor
                        nc.gpsimd.tensor_mul(
                            tmp[:, d, :], p1t[:, d, :],
                            ov[:, d:d + 1].broadcast_to([128, N]))
                nc.vector.tensor_add(tr1[:, 0:8, :], tmp[:, 0:8, :],
                                     tmp[:, 8:16, :])
                nc.vector.tensor_add(tmp[:, 0:4, :], tr1[:, 0:4, :],
                                     tr1[:, 4:8, :])
                nc.vector.tensor_add(tr1[:, 0:2, :], tmp[:, 0:2, :],
                                     tmp[:, 2:4, :])
                if lg_prev is None:
                    nc.vector.tensor_add(lg_out[:], tr1[:, 0, :], tr1[:, 1, :])
                else:
                    nc.vector.tensor_add(tr1[:, 2, :], tr1[:, 0, :],
                                         tr1[:, 1, :])
                    nc.vector.tensor_add(lg_out[:], tr1[:, 2, :], lg_prev[:])
                return tmp, tr1

            def s_side_a(p1t, e, tmp, tr1):
                """sr[:,d] = sum_n e * p1t[:,d,:]: one 2x broadcast mul +
                halving add tree over n, reusing g-side scratch."""
                w2 = tp.tile([128, D, N], F16, tag="w2")
                nc.vector.tensor_mul(
                    w2[:], p1t[:],
                    e[:, None, :].broadcast_to([128, D, N]))
                nc.vector.tensor_add(tmp[:, :, 0:144], w2[:, :, 0:144],
                                     w2[:, :, 144:288])
                nc.vector.tensor_add(w2[:, :, 0:72], tmp[:, :, 0:72],
                                     tmp[:, :, 72:144])
                nc.vector.tensor_add(tmp[:, :, 0:36], w2[:, :, 0:36],
                                     w2[:, :, 36:72])
                nc.vector.tensor_add(w2[:, :, 0:18], tmp[:, :, 0:18],
                                     tmp[:, :, 18:36])
                sr = sp.tile([128, D], F32, tag="sr")
                nc.vector.tensor_reduce(out=sr[:], in_=w2[:, :, 0:18],
                                        axis=AX.X, op=AL.add)
                return sr

            def s_side_c(u, segs, e, psv, slot):
                """s numerator on the PE: EX = e*xw (one 2x DVE mul), PE
                transposes EX chunks through PSUM (GPSIMD copies them back
                as fp16 stationaries), then 18 shared-weight matmuls
                accumulate sum_{n,c} EX^T * W into psv[:, slot, :]."""
                slab = xws[:, _slab_slot(u)]
                ex = tp.tile([128, CH, Cin, G], F16, tag="ex")
                for (a, p0, ro, cnt) in segs:
                    nc.vector.tensor_mul(
                        ex[ro:ro + cnt], slab[ro:ro + cnt],
                        e[ro:ro + cnt]
                        .rearrange("p (c2 g) -> p c2 g", g=G)[:, :, None, :]
                        .broadcast_to([cnt, CH, Cin, G]))
                exts = tp.tile([128, CH, 128], F16, tag="exts")
                if S_MODE == "d":
                    # transposes in two bank-aligned PSUM groups of 9, each
                    # drained by one 2x DVE copy
                    for gi in range(2):
                        exq_t = qx.tile([128, 16, 128], F16, tag="exq")
                        for cl in range(9):
                            nc.tensor.transpose(
                                exq_t[:, cl, :],
                                ex[:, gi * 9 + cl].rearrange("p c g -> p (c g)"),
                                ident[:])
                        nc.vector.tensor_copy(
                            exts[:, gi * 9:(gi + 1) * 9, :],
                            exq_t[:, 0:9, :])
                else:
                    for gi in range(3):
                        exq_t = qx.tile([128, 8, 128], F16, tag="exq")
                        for cl in range(6):
                            ch = gi * 6 + cl
                            nc.tensor.transpose(
                                exq_t[:, cl, :],
                                ex[:, ch].rearrange("p c g -> p (c g)"),
                                ident[:])
                        nc.scalar.copy(exts[:, gi * 6:(gi + 1) * 6, :],
                                       exq_t[:, 0:6, :])
                for (a, p0, ro, cnt) in segs:
                    for ch in range(CH):
                        nc.tensor.matmul(
                            psv[ro:ro + cnt, slot, :],
                            exts[:, ch, ro:ro + cnt],
                            wst[:, a, ch, :],
                            start=(ch == 0), stop=(ch == CH - 1))
                return psv[:, slot, :]

            def routing_gen(u, p1t, psv):
                segs = _segments(u)
                bu = bunit[:, u, :]
                # ---- iter 0 (uniform probs; s0 from the PE chain)
                s = sp.tile([128, D], F32, tag="s0")
                nc.vector.scalar_tensor_tensor(
                    out=s[:], in0=psv[:, 0, :], scalar=1.0 / N, in1=bu,
                    op0=AL.mult, op1=AL.add)
                sn, r = squash_pre(s, "0")
                yield
                rt = squash_act(sn, "0")
                yield
                ov = squash_post(s, rt, r, "0")

                lg_prev = None
                for it in (1, 2):
                    sq = str(it)
                    lg = lp.tile([128, N], F16, tag="lg" + sq)
                    tmp, tr1 = g_chain(p1t, ov, lg, lg_prev)
                    lg_prev = lg
                    nmx = sp.tile([128, 1], F32, tag="nmx")
                    nc.vector.tensor_reduce(out=nmx[:], in_=lg[:], axis=AX.X,
                                            op=AL.max, negate=True)
                    yield
                    e = sp.tile([128, N], F16, tag="e")
                    se = sp.tile([128, 1], F32, tag="se")
                    nc.scalar.activation(e[:], lg[:], AF.Exp, bias=nmx[:],
                                         scale=1.0, accum_out=se[:])
                    yield
                    rc = sp.tile([128, 1], F32, tag="rc")
                    nc.vector.reciprocal(rc[:], se[:])
                    if S_MODE in ("c", "d"):
                        sr = s_side_c(u, segs, e, psv, it)
                    else:
                        sr = s_side_a(p1t, e, tmp, tr1)
                    s = sp.tile([128, D], F32, tag="s" + sq)
                    nc.vector.scalar_tensor_tensor(
                        out=s[:], in0=sr[:], scalar=rc[:], in1=bu,
                        op0=AL.mult, op1=AL.add)
                    sn, r = squash_pre(s, sq)
                    yield
                    rt = squash_act(sn, sq)
                    yield
                    ov = squash_post(s, rt, r, sq)

                for (a, p0, ro, cnt) in segs:
                    nc.sync.dma_start(out_d[a, p0:p0 + cnt, :],
                                      ov[ro:ro + cnt, :])
                yield

            # ---- emit: phase A one pair ahead (keeps the PE from head-of-
            # line blocking behind routing-side PE work), routing zipped
            # across each pair so DVE never stalls on ACT/PE round-trips.
            pa = {}

            def ensure_pa(u):
                if u < NU and u not in pa:
                    pa[u] = phase_a(u)

            ensure_pa(0)
            ensure_pa(1)
            for j in range(0, NU, 2):
                ensure_pa(j + 2)
                ensure_pa(j + 3)
                g0 = routing_gen(j, *pa[j])
                g1 = routing_gen(j + 1, *pa[j + 1])
                # stagger the pair by two stages so the two units hit their
                # cross-engine waits out of phase (a zipped pair otherwise
                # stalls simultaneously)
                next(g0)
                next(g0)
                alive = [g0, g1]
                while alive:
                    nxt = []
                    for g in alive:
                        try:
                            next(g)
                            nxt.append(g)
                        except StopIteration:
                            pass
                    alive = nxt

    nc.finalize()
    _prog_cache[key] = nc
    return nc


def _host_prep(x, route_weights, bias):
    x = np.ascontiguousarray(x, dtype=np.float32)
    Wfull = np.ascontiguousarray(route_weights, dtype=np.float32)
    bias = np.ascontiguousarray(bias, dtype=np.float32)

    # im2col: xw[p, n, c], node ordering (ci, ki, kj) as in torch .view
    xw = np.empty((B, w, w, Ci, K, K, Cin), np.float32)
    for ki in range(K):
        for kj in range(K):
            xw[:, :, :, :, ki, kj, :] = (
                x[:, :, ki:ki + w, kj:kj + w, :].transpose(0, 2, 3, 1, 4))
    xw = xw.reshape(P, N, Cin)

    # xwt[(g,c), ch, p]
    xw4 = xw.reshape(P, CH, G, Cin)
    xwt_h = np.ascontiguousarray(
        xw4.transpose(2, 3, 1, 0).reshape(128, CH, P)).astype(np.float16)

    Wn = Wfull.reshape(A, CH, G, Cin, D)
    # block-diag moving weights, (d, g)-major columns:
    # wbd[a, (g,c), ch, (d, g')] = W[a, ch, g, c, d] iff g' == g
    wbd_full = np.zeros((A, G, Cin, CH, D, G), np.float32)
    for g in range(G):
        # Wn[:, :, g, :, :]: [A, CH, Cin, D] -> [A, Cin, CH, D]
        wbd_full[:, g, :, :, :, g] = Wn[:, :, g, :, :].transpose(0, 2, 1, 3)
    wbd_h = wbd_full.reshape(A, 128, CH, D * G).astype(np.float16)

    # wde[(g,c), a, ch, d]
    wde_h = np.ascontiguousarray(
        Wn.transpose(2, 3, 0, 1, 4).reshape(128, A, CH, D)).astype(np.float16)

    if S_MODE in ("c", "d"):
        # xw in (p, ch, c, g) order, staged at 9 partition alignments so any
        # unit's 128 rows are one contiguous slab slice
        xw_cng = np.ascontiguousarray(
            xw.reshape(P, CH, G, Cin).transpose(0, 1, 3, 2))  # [P, CH, c, g]
        rows = np.arange(128)
        xws_h = np.zeros((128, 9, CH, Cin, G), np.float16)
        for q in range(4):
            xws_h[:, q] = xw_cng[128 * q + rows]
        for kk in range(4):
            xws_h[:, 4 + kk] = xw_cng[64 + 128 * kk + rows]
        xws_h[:, 8] = xw_cng[(512 + rows) % P]
        # wst[(c,g), a, ch, d] — (c,g) row order matches EX's flattening
        wst_h = np.ascontiguousarray(
            Wn.transpose(3, 2, 0, 1, 4).reshape(128, A, CH, D)).astype(np.float16)

    in_maps = []
    for k in range(8):
        a0 = k * AA
        # per-unit per-row bias: rows are flattened (a_local, p)
        bunit_h = np.empty((128, NU, D), np.float32)
        for u in range(NU):
            rows = np.arange(u * 128, u * 128 + 128)
            bunit_h[:, u, :] = bias[a0 + rows // P]
        im = {
            "xwt": xwt_h,
            "wbd": np.ascontiguousarray(wbd_h[a0:a0 + AA]),
            "wde": np.ascontiguousarray(wde_h[:, a0:a0 + AA]),
            "bunit": bunit_h,
        }
        if S_MODE in ("c", "d"):
            im["xws"] = xws_h
            im["wst"] = np.ascontiguousarray(wst_h[:, a0:a0 + AA])
        in_maps.append(im)
    return in_maps


def kernel(x, route_weights, bias):
    global LAST_RESULT
    nc = _build_program()
    in_maps = _host_prep(x, route_weights, bias)
    trace = bool(os.environ.get("KERNEL_TRACE"))
    res = run_bass_kernel_spmd(nc, in_maps, list(range(8)), trace=trace)
    LAST_RESULT = res
    full = np.stack([res.results[k]["out"] for k in range(8)])  # [8, AA, P, D]
    full = full.reshape(A, B, w, w, D)
    return np.ascontiguousarray(full.transpose(1, 0, 2, 3, 4))
